# revision 22
# baseline (speedup 1.0000x reference)
"""BiLevelRoutingAttention Trainium2 kernel.

Sharding: data-parallel over (T*B)=8 cores; core = b*4 + t.
Host: windowize + transpose + region-routing top-k (0.005% of FLOPs).
Device: qkv projection (fp32), LIF spike bits, per-window gathered
kv/ksum contractions (bf16 bits, exact integer arithmetic), linear
attention with fused denominator column, output projection (fp32).
The top-k window indices (which depend only on batch b) are baked into
the program; cores select their variant via tc.If(partition_id).
"""

import os
import numpy as np

# problem constants (hardcoded per contract)
T, B, Lt, Lh, Lw, C = 4, 2, 8, 32, 32, 256
WT, WH, WW = 4, 4, 4
NW = WT * WH * WW              # 64 windows
PT, PH, PW = Lt // WT, Lh // WH, Lw // WW
WS = PT * PH * PW              # 128 tokens per window
H, HD = 8, C // 8
TOPK = 4
NTOK = NW * WS                 # 8192 tokens per (t,b) shard
N_CORES = 8

last_results = None            # stashed BassKernelResults for test harness
last_nc = None
last_in_maps = None


def _windowize(x):
    xw = x.reshape(T, B, WT, PT, WH, PH, WW, PW, C)
    xw = xw.transpose(0, 1, 2, 4, 6, 3, 5, 7, 8).reshape(T, B, NW, WS, C)
    return xw


def _unwindowize(ow):
    o = ow.reshape(T, B, WT, WH, WW, PT, PH, PW, C)
    o = o.transpose(0, 1, 2, 5, 3, 6, 4, 7, 8).reshape(T, B, Lt, Lh, Lw, C)
    return o


def _routing_idx(xw32):
    """Mimic reference routing in fp32: region scores -> top-4 window idx."""
    region = xw32.sum(0).mean(2)                           # [B,NW,C]
    scores = np.einsum('bic,bjc->bij', region, region) * np.float32(HD ** -0.5)
    # jax.lax.top_k tie-break = lowest index first; stable argsort matches
    idx = np.argsort(-scores, axis=-1, kind='stable')[:, :, :TOPK]
    return idx                                             # [B,NW,TOPK]


def _build_program(idx_by_b, debug=False):
    import concourse.bass as bass
    import concourse.mybir as mybir
    import concourse.tile as tile
    from concourse import bacc
    from concourse.masks import make_identity

    f32 = mybir.dt.float32
    f16 = mybir.dt.float16
    bf16 = mybir.dt.bfloat16

    nc = bacc.Bacc("TRN2", target_bir_lowering=False, debug=False,
                   num_devices=N_CORES)

    xwT = nc.dram_tensor("xwT", [C, NTOK], f32, kind="ExternalInput").ap()
    wq = nc.dram_tensor("wq", [C, 3 * C], f32, kind="ExternalInput").ap()
    bq = nc.dram_tensor("bq", [3 * C], f32, kind="ExternalInput").ap()
    wp = nc.dram_tensor("wp", [C, C], f32, kind="ExternalInput").ap()
    bp = nc.dram_tensor("bp", [C], f32, kind="ExternalInput").ap()
    selk = nc.dram_tensor("selk", [2], f32, kind="ExternalInput").ap()
    masks = nc.dram_tensor("masks", [128, 528], f32, kind="ExternalInput").ap()
    out_d = nc.dram_tensor("out", [NTOK, C], f32, kind="ExternalOutput").ap()
    if debug:
        dbg_q = nc.dram_tensor("dbg_q", [128, NW * 256], mybir.dt.bfloat16, kind="ExternalOutput").ap()
        dbg_k = nc.dram_tensor("dbg_k", [128, NW * 256], mybir.dt.bfloat16, kind="ExternalOutput").ap()
        dbg_v = nc.dram_tensor("dbg_v", [128, NW * 260], mybir.dt.bfloat16, kind="ExternalOutput").ap()
        dbg_kvs = nc.dram_tensor("dbg_kvs", [64, 260], mybir.dt.float16, kind="ExternalOutput").ap()
        dbg_qTw = nc.dram_tensor("dbg_qTw", [64, 512], mybir.dt.float16, kind="ExternalOutput").ap()
        dbg_at = nc.dram_tensor("dbg_at", [128, 256], f32, kind="ExternalOutput").ap()
        dbg_dr = nc.dram_tensor("dbg_dr", [128, 8], f32, kind="ExternalOutput").ap()

    with tile.TileContext(nc) as tc:
        with (
            tc.tile_pool(name="const", bufs=1) as const_pool,
            tc.tile_pool(name="bits", bufs=1) as bits_pool,
            tc.tile_pool(name="xt", bufs=4) as xt_pool,
            tc.tile_pool(name="work", bufs=3) as work_pool,
            tc.tile_pool(name="tpsum", bufs=2, space="PSUM") as tpsum,
        ):
            # ---- resident constants ----
            wq_sb = const_pool.tile([128, 2 * 768], f32, tag="wq")
            for kc in range(2):
                nc.sync.dma_start(wq_sb[:, kc * 768:(kc + 1) * 768],
                                  wq[kc * 128:(kc + 1) * 128, :])
            wp_sb = const_pool.tile([128, 2 * 256], f32, tag="wp")
            for kc in range(2):
                nc.sync.dma_start(wp_sb[:, kc * 256:(kc + 1) * 256],
                                  wp[kc * 128:(kc + 1) * 128, :])
            ident_b = const_pool.tile([128, 128], bf16, tag="idb")
            make_identity(nc, ident_b)
            ident_f = const_pool.tile([128, 128], f32, tag="idf")
            make_identity(nc, ident_f)

            ones_row = const_pool.tile([1, 128], f32, tag="ones")
            nc.vector.memset(ones_row, 1.0)
            bq_row = const_pool.tile([1, 768], f32, tag="bqr")
            nc.sync.dma_start(bq_row, bq[None, :])
            bp_row = const_pool.tile([1, 256], f32, tag="bpr")
            nc.sync.dma_start(bp_row, bp[None, :])
            sel_row = const_pool.tile([1, 2], f32, tag="selr")
            nc.sync.dma_start(sel_row, selk[None, :])
            sel_bc = const_pool.tile([128, 2], f32, tag="selbc")
            mask_sb = const_pool.tile([128, 528], f32, tag="masks")
            nc.sync.dma_start(mask_sb, masks)

            thr = const_pool.tile([128, 768], f32, tag="thr")
            bp_bc = const_pool.tile([128, 256], f32, tag="bpbc")

            # ---- bit tensors (resident) ----
            q_bits = bits_pool.tile([128, NW * 256], bf16, tag="qb")
            k_bits0 = bits_pool.tile([128, NW * 256], bf16, tag="kb0")
            k_bits1 = bits_pool.tile([128, NW * 256], bf16, tag="kb1")
            v_ext = bits_pool.tile([128, NW * 257], bf16, tag="vb")
            v_r = v_ext.rearrange("p (w d) -> p w d", d=257)
            nc.vector.memset(v_r[:, :, 256], 1.0)

            # ---- stage 1: qkv projection + LIF + q transpose ----
            with tc.tile_pool(name="qkv_ps", bufs=2, space="PSUM") as qkv_psum:
                # broadcast bias rows across partitions via ones-column matmul
                bc_ps = qkv_psum.tile([128, 768], f32, tag="qkv")
                nc.tensor.matmul(bc_ps[:, 0:512], ones_row, bq_row[:, 0:512],
                                 start=True, stop=True)
                nc.tensor.matmul(bc_ps[:, 512:768], ones_row,
                                 bq_row[:, 512:768], start=True, stop=True)
                # thr = 2 - b_qkv  (spike(x) fires iff qkv + b >= 2)
                nc.vector.tensor_scalar(out=thr[:, 0:512], in0=bc_ps[:, 0:512],
                                        scalar1=-1.0, scalar2=2.0,
                                        op0=mybir.AluOpType.mult,
                                        op1=mybir.AluOpType.add)
                nc.vector.tensor_scalar(out=thr[:, 512:768],
                                        in0=bc_ps[:, 512:768],
                                        scalar1=-1.0, scalar2=2.0,
                                        op0=mybir.AluOpType.mult,
                                        op1=mybir.AluOpType.add)
                bc_ps2 = qkv_psum.tile([128, 768], f32, tag="qkv")
                nc.tensor.matmul(bc_ps2[:, 0:256], ones_row, bp_row,
                                 start=True, stop=True)
                nc.tensor.matmul(bc_ps2[:, 256:258], ones_row, sel_row,
                                 start=True, stop=True)
                nc.scalar.copy(bp_bc, bc_ps2[:, 0:256])
                nc.scalar.copy(sel_bc, bc_ps2[:, 256:258])
                # variant thresholds for k bits: +1e30 disables spikes on
                # cores of the other batch, so wrong-variant gather terms
                # contribute exactly zero
                thrk0 = const_pool.tile([128, 256], f32, tag="thrk0")
                thrk1 = const_pool.tile([128, 256], f32, tag="thrk1")
                nc.vector.tensor_scalar_add(thrk0, thr[:, 256:512],
                                            sel_bc[:, 0:1])
                nc.vector.tensor_scalar_add(thrk1, thr[:, 256:512],
                                            sel_bc[:, 1:2])
                for n in range(NW):
                    xt0 = xt_pool.tile([128, 128], f32, tag="xt")
                    xt1 = xt_pool.tile([128, 128], f32, tag="xt")
                    nc.sync.dma_start(xt0, xwT[0:128, n * 128:(n + 1) * 128])
                    nc.sync.dma_start(xt1, xwT[128:256, n * 128:(n + 1) * 128])
                    ps = qkv_psum.tile([128, 768], f32, tag="qkv")
                    nc.tensor.matmul(ps[:, 0:512], xt0, wq_sb[:, 0:512],
                                     start=True, stop=False)
                    nc.tensor.matmul(ps[:, 0:512], xt1, wq_sb[:, 768:1280],
                                     start=False, stop=True)
                    nc.tensor.matmul(ps[:, 512:768], xt0, wq_sb[:, 512:768],
                                     start=True, stop=False)
                    nc.tensor.matmul(ps[:, 512:768], xt1, wq_sb[:, 1280:1536],
                                     start=False, stop=True)
                    # LIF spike bits: (qkv + b >= 2) == (matmul >= thr)
                    nc.vector.tensor_tensor(
                        out=q_bits[:, n * 256:(n + 1) * 256],
                        in0=ps[:, 0:256], in1=thr[:, 0:256],
                        op=mybir.AluOpType.is_ge)
                    nc.vector.tensor_tensor(
                        out=k_bits0[:, n * 256:(n + 1) * 256],
                        in0=ps[:, 256:512], in1=thrk0,
                        op=mybir.AluOpType.is_ge)
                    nc.vector.tensor_tensor(
                        out=k_bits1[:, n * 256:(n + 1) * 256],
                        in0=ps[:, 256:512], in1=thrk1,
                        op=mybir.AluOpType.is_ge)
                    nc.vector.tensor_tensor(
                        out=v_r[:, n, 0:256],
                        in0=ps[:, 512:768], in1=thr[:, 512:768],
                        op=mybir.AluOpType.is_ge)

            # ---- stage 2: routed attention + projection ----
            def attention_stage(_unused):
                with (
                    tc.tile_pool(name="kv_ps", bufs=2, space="PSUM") as kv_psum,
                    tc.tile_pool(name="at_ps", bufs=2, space="PSUM") as at_psum,
                    tc.tile_pool(name="pj_ps", bufs=2, space="PSUM") as pj_psum,
                ):
                    for n in range(NW):
                        kv0 = kv_psum.tile([128, 257], f32, tag="kv")
                        kv1 = kv_psum.tile([128, 257], f32, tag="kv")
                        mm = 0
                        for vi, (kb_v, idxv) in enumerate(
                                [(k_bits0, idx_by_b[0]), (k_bits1, idx_by_b[1])]):
                            js = [int(j) for j in idxv[n]]
                            for jj, j in enumerate(js):
                                st = vi == 0 and jj == 0
                                sp = vi == 1 and jj == 3
                                nc.tensor.matmul(
                                    kv0, kb_v[:, j * 256:j * 256 + 128],
                                    v_ext[:, j * 257:(j + 1) * 257],
                                    start=st, stop=sp)
                                nc.tensor.matmul(
                                    kv1, kb_v[:, j * 256 + 128:(j + 1) * 256],
                                    v_ext[:, j * 257:(j + 1) * 257],
                                    start=st, stop=sp)
                        # masked copy -> block-diagonal kv + per-head ksum cols
                        kvs = work_pool.tile([128, 528], f16, tag="kvs")
                        for hf, kvh in enumerate([kv0, kv1]):
                            nc.vector.tensor_tensor(
                                out=kvs[:, hf * 264:hf * 264 + 256],
                                in0=kvh[:, 0:256],
                                in1=mask_sb[:, hf * 264:hf * 264 + 256],
                                op=mybir.AluOpType.mult)
                            nc.vector.tensor_tensor(
                                out=kvs[:, hf * 264 + 256:hf * 264 + 264],
                                in0=kvh[:, 256:257].to_broadcast([128, 8]),
                                in1=mask_sb[:, hf * 264 + 256:hf * 264 + 264],
                                op=mybir.AluOpType.mult)
                        # transpose q bits -> [c, s]
                        qT_w = work_pool.tile([128, 256], f16, tag="qTw")
                        for hf in range(2):
                            tp = tpsum.tile([128, 128], bf16, tag="tr")
                            nc.tensor.transpose(
                                tp,
                                q_bits[:, n * 256 + hf * 128:n * 256 + (hf + 1) * 128],
                                ident_b)
                            nc.scalar.copy(
                                qT_w[:, hf * 128:(hf + 1) * 128], tp)
                        # numerator + per-head D in one K=128 pair
                        ap_ = at_psum.tile([128, 264], f32, tag="at")
                        nc.tensor.matmul(ap_, qT_w[:, 0:128],
                                         kvs[:, 0:264], start=True, stop=False)
                        nc.tensor.matmul(ap_, qT_w[:, 128:256],
                                         kvs[:, 264:528], start=False, stop=True)
                        dr = work_pool.tile([128, 8], f32, tag="dr")
                        nc.vector.tensor_scalar_add(dr, ap_[:, 256:264], 1e-6)
                        nc.vector.reciprocal(dr, dr)
                        at = work_pool.tile([128, 256], f32, tag="attn")
                        for h in range(H):
                            nc.vector.tensor_scalar_mul(
                                at[:, h * 32:(h + 1) * 32],
                                ap_[:, h * 32:(h + 1) * 32],
                                dr[:, h:h + 1])
                        aT = work_pool.tile([128, 256], f32, tag="aT")
                        for kd in range(2):
                            tp = tpsum.tile([128, 128], f32, tag="tr")
                            nc.tensor.transpose(
                                tp, at[:, kd * 128:(kd + 1) * 128], ident_f)
                            nc.scalar.copy(aT[:, kd * 128:(kd + 1) * 128], tp)
                        pp = pj_psum.tile([128, 256], f32, tag="pj")
                        nc.tensor.matmul(pp, aT[:, 0:128], wp_sb[:, 0:256],
                                         start=True, stop=False)
                        nc.tensor.matmul(pp, aT[:, 128:256], wp_sb[:, 256:512],
                                         start=False, stop=True)
                        ob = work_pool.tile([128, 256], f32, tag="ob")
                        nc.vector.tensor_tensor(out=ob, in0=pp, in1=bp_bc,
                                                op=mybir.AluOpType.add)
                        nc.sync.dma_start(out_d[n * 128:(n + 1) * 128, :], ob)

            if debug:
                nc.sync.dma_start(dbg_q, q_bits)
                nc.sync.dma_start(dbg_k, k_bits0)
                nc.sync.dma_start(dbg_v, v_ext)
            attention_stage(None)

    nc.compile()
    return nc


def kernel(x, W_qkv, b_qkv, W_proj, b_proj):
    global last_results
    from concourse import bass_utils

    x = np.asarray(x, dtype=np.float32)
    xw = _windowize(x)                                     # [T,B,NW,WS,C]
    idx = _routing_idx(xw)                                 # [B,NW,TOPK]

    nc = _build_program(idx)

    mask = np.zeros((128, 528), np.float32)
    for hf in range(2):
        for cr in range(128):
            h = hf * 4 + cr // 32                  # global head of row cr
            mask[cr, hf * 264 + h * 32:hf * 264 + (h + 1) * 32] = 1.0
            mask[cr, hf * 264 + 256 + h] = 1.0

    in_maps = []
    for core in range(N_CORES):
        b, t = divmod(core, T)
        xwT_c = np.ascontiguousarray(
            xw[t, b].reshape(NTOK, C).T)                   # [C, NTOK]
        sel = np.array([0.0, 1e30] if b == 0 else [1e30, 0.0], np.float32)
        in_maps.append({
            "xwT": xwT_c,
            "selk": sel,
            "masks": mask,
            "wq": np.asarray(W_qkv, np.float32),
            "bq": np.asarray(b_qkv, np.float32),
            "wp": np.asarray(W_proj, np.float32),
            "bp": np.asarray(b_proj, np.float32),
        })

    res = bass_utils.run_bass_kernel_spmd(
        nc, in_maps, core_ids=list(range(N_CORES)), trace=False)
    last_results = res
    global last_nc, last_in_maps
    last_nc, last_in_maps = nc, in_maps

    ow = np.empty((T, B, NW, WS, C), np.float32)
    for core in range(N_CORES):
        b, t = divmod(core, T)
        ow[t, b] = res.results[core]["out"].reshape(NW, WS, C)
    return _unwindowize(ow)
